# revision 18
# baseline (speedup 1.0000x reference)
"""AdMSoftmaxLoss fused distributed kernel for 8 TRN2 NeuronCores (v3).

Math (reference):
    xn = x / ||x||                     # row-L2-normalized embeddings
    wf = xn @ W.T                      # [N, C] logits
    tgt = wf[i, y_i]
    num = S * (tgt - M)
    excl = sum_c exp(S*wf) - exp(S*tgt)
    L = num - log(exp(num) + excl);  loss = -mean(L)

Strategy: pure data-parallel over N (2048 rows/core), no collectives.
The scale S/||x_i|| is folded into x on the HOST (xs = S*x/||x||), so the
device matmul produces final logits directly and needs no per-row scale.
  - PE: fp8e4 DoubleRow matmuls (K=256 per instruction; W pre-scaled by 16
    on the host for fp8 range, the 1/16 folded into the consumers).
  - The exp+row-sum work (20.5M elements/core) is SPLIT between the two
    engines that can read PSUM, each with its own PRIVATE PSUM ring so the
    rings self-pipeline with no cross-engine coupling:
      * ScalarE (ACT): exp activation with scale=1/16 and accum_out over
        double-buffered [128,1536] slots (6 banks);
      * VectorE (DVE): Schraudolph bit-trick exp over one [128,1024] slot
        (2 banks): tensor_scalar affine fp32(PSUM) -> int16 bf16-bits
        (round-to-nearest), then one scalar_tensor_tensor that adds the
        two bf16 halves and accum-sums the fp32 row total while the PE
        refills the slot.
    Schraudolph bf16 has ~+-4% sawtooth error, bias-corrected via the
    magic constant; per-row sums average it to ~0.3%, negligible vs the
    2e-2 gate.
  - Target logits S*tgt come from DVE dot products of bf16 xs rows with
    the host-gathered g = W[labels] rows (accum_out).
  - Final log via the DVE exponent/poly bit-trick (no ACT table reload),
    done per tile-half so only half the tail is exposed.
Per-row L values are DMA'd out; the host concatenates and means.
"""

import numpy as np
import ml_dtypes

import concourse.mybir as mybir
import concourse.tile as tile
from concourse import bacc
from concourse.bass_utils import run_bass_kernel_spmd

N, D, C = 16384, 256, 10000
S, M = 30.0, 0.4
NCORES = 8
NS = N // NCORES      # 2048 rows per core
NT = NS // 128        # 16 n-tiles of 128 rows
KT = D // 128         # 2 k-slices (one DoubleRow pass)

_F32 = mybir.dt.float32
_BF16 = mybir.dt.bfloat16
_I16 = mybir.dt.int16
_I32 = mybir.dt.int32
_F8 = mybir.dt.float8e4

LN2 = float(np.log(2.0))
WSCALE = 16.0                       # host pre-scale on W for fp8 range
A16 = 128.0 / LN2 / WSCALE          # Schraudolph slope on 16x logits
B16 = 16256.0 - 7.37                # bf16 magic offset, mean-unbiased
SM = S * M

N_WARMUP_MM = 10
AW = 1536                           # ACT ring slot width (3 banks x 2 bufs)
DW = 1024                           # DVE ring slot width (2 banks x 1 buf)

# Per-tile chunk layouts: (engine, col0, width) with 8 chunks per tile.
_LAYOUT_D4 = (
    [("A", 0, 1536), ("A", 1536, 1536), ("A", 3072, 1536), ("A", 4608, 1296)]
    + [("D", 5904 + i * 1024, 1024) for i in range(4)]
)
_LAYOUT_D3 = (
    [("A", 0, 1536), ("A", 1536, 1536), ("A", 3072, 1536), ("A", 4608, 1536),
     ("A", 6144, 784)]
    + [("D", 6928 + i * 1024, 1024) for i in range(3)]
)
# Head tiles: DVE columns sit right after A0 so the first DVE fills only
# need the first two wt DMA pieces (cols < 5632) and the DVE stream can
# start ~15us earlier.
_LAYOUT_HEAD = (
    [("A", 0, 1536)]
    + [("D", 1536 + i * 1024, 1024) for i in range(4)]
    + [("A", 5632, 1536), ("A", 7168, 1536), ("A", 8704, 1296)]
)
NCH = 8                             # chunks (esum slots) per tile


def _tile_layout(t):
    """5904/4096 (4 DVE chunks) or 6928/3072 (3 DVE chunks) col split."""
    if t < 2:
        return _LAYOUT_HEAD
    return _LAYOUT_D3 if (t % 8 in (2, 5, 7) or t == 6) else _LAYOUT_D4


def _build_nc(ns=NS, c=C):
    nt = ns // 128
    nc = bacc.Bacc("TRN2", target_bir_lowering=False)
    AF = mybir.ActivationFunctionType
    NT, C = nt, c  # noqa: N806
    NS = ns  # noqa: N806
    NH = NT // 2  # noqa: N806
    DR = mybir.MatmulPerfMode.DoubleRow  # noqa: N806
    mult = mybir.AluOpType.mult
    sub = mybir.AluOpType.subtract
    addop = mybir.AluOpType.add

    xt_ext = nc.declare_dram_parameter("xt", [128, KT, NS], _F8, isOutput=False)
    wt_ext = nc.declare_dram_parameter("wt", [128, KT, C], _F8, isOutput=False)
    xf_ext = nc.declare_dram_parameter("xf", [128, NT, D], _BF16, isOutput=False)
    g_ext = nc.declare_dram_parameter("g", [128, NT, D], _BF16, isOutput=False)
    out_ext = nc.declare_dram_parameter("out", [128, NT], _F32, isOutput=True)

    with tile.TileContext(nc) as tc:
        with (
            tc.tile_pool(name="big", bufs=1) as big,
            tc.tile_pool(name="stat", bufs=1) as stat,
            tc.tile_pool(name="scr", bufs=1) as scr,
            tc.tile_pool(name="expb", bufs=4) as expb,
            tc.tile_pool(name="ybuf", bufs=3) as ybuf,
            tc.tile_pool(name="dsum", bufs=2) as dsum,
            tc.tile_pool(name="psa", bufs=2, space="PSUM") as psa,
            tc.tile_pool(name="psd", bufs=1, space="PSUM") as psd,
        ):
            # ---- prologue: warm the exp ACT table + PE pstate during DMAs ----
            wu_a = scr.tile([128, KT, 128], _F8)
            wu_b = scr.tile([128, KT, 512], _F8)
            wu_e = scr.tile([128, 1], _F32)
            nc.gpsimd.memset(wu_a, 0.0)
            nc.gpsimd.memset(wu_b, 0.0)
            nc.gpsimd.memset(wu_e, 0.0)
            nc.scalar.activation(wu_e, wu_e, AF.Exp)  # pull exp table load
            wu_p = psa.tile([128, AW], _F32, tag="pa")
            for i in range(N_WARMUP_MM):
                nc.tensor.matmul(
                    wu_p[:, (i % 3) * 512 : (i % 3) * 512 + 512],
                    wu_a,
                    wu_b,
                    start=True,
                    stop=True,
                    perf_mode=DR,
                )

            # ---- input DMAs, ordered by when they gate compute ----
            xf_sb = big.tile([128, NT, D], _BF16)
            g_sb = big.tile([128, NT, D], _BF16)
            wt_sb = big.tile([128, KT, C], _F8)
            xt_sb = big.tile([128, KT, NS], _F8)

            def _wt_chunk(c0, w):
                for k in range(KT):
                    nc.sync.dma_start(
                        out=wt_sb[:, k, c0 : c0 + w], in_=wt_ext[:, k, c0 : c0 + w]
                    )

            _wt_chunk(0, 2048)
            for k in range(KT):
                nc.sync.dma_start(out=xt_sb[:, k, :], in_=xt_ext[:, k, :])
            _wt_chunk(2048, 2048)
            nc.sync.dma_start(out=xf_sb[:, :NH, :], in_=xf_ext[:, :NH, :])
            _wt_chunk(4096, 2048)
            _wt_chunk(6144, 2048)
            nc.sync.dma_start(out=g_sb[:, :NH, :], in_=g_ext[:, :NH, :])
            _wt_chunk(8192, 1808)
            nc.sync.dma_start(out=xf_sb[:, NH:, :], in_=xf_ext[:, NH:, :])
            nc.sync.dma_start(out=g_sb[:, NH:, :], in_=g_ext[:, NH:, :])

            # ---- per-(tile, chunk) partial exp-sums ----
            esum_a = stat.tile([128, NH * NCH], _F32)
            esum_b = stat.tile([128, NH * NCH], _F32)
            esum_h = [esum_a, esum_b]

            rawt = stat.tile([128, NT], _F32)   # S * tgt
            num = stat.tile([128, NT], _F32)    # S * (tgt - M)
            prod = scr.tile([128, NT, D], _BF16)  # xf*g products (gpsimd)

            def _slot(t, ci):
                h, th = (0, t) if t < NH else (1, t - NH)
                return esum_h[h], th * NCH + ci

            def _fill(t, c0, w, pool, tag, width):
                pt = pool.tile([128, width], _F32, tag=tag)
                for b0 in range(0, w, 512):
                    bw = min(512, w - b0)
                    nc.tensor.matmul(
                        pt[:, b0 : b0 + bw],
                        xt_sb[:, :, t * 128 : (t + 1) * 128],
                        wt_sb[:, :, c0 + b0 : c0 + b0 + bw],
                        start=True,
                        stop=True,
                        perf_mode=DR,
                    )
                return pt

            def _act_chunk(t, ci, c0, w):
                pt = _fill(t, c0, w, psa, "pa", AW)
                esum, idx = _slot(t, ci)
                eo = expb.tile([128, AW], _BF16, tag="eo")
                nc.scalar.activation(
                    eo[:, :w],
                    pt[:, :w],
                    AF.Exp,
                    scale=1.0 / WSCALE,
                    accum_out=esum[:, idx : idx + 1],
                )

            def _dve_chunk(t, ci, c0, w):
                pt = _fill(t, c0, w, psd, "pd", DW)
                esum, idx = _slot(t, ci)
                y = ybuf.tile([128, DW], _I16, tag="y")
                # pass 1: i16 = rne(A16 * z16 + B16); bitcast(i16) ~ exp(z)
                nc.vector.tensor_scalar(
                    y[:, :w], pt[:, :w], A16, B16, mult, addop
                )
                yb = y.bitcast(_BF16)
                h2 = w // 2
                ds = dsum.tile([128, DW // 2], _BF16, tag="ds")
                # pass 2: halves-add + accumulate the fp32 row sum; the PE
                # refills the (already released) slot under this op
                nc.vector.scalar_tensor_tensor(
                    out=ds[:, :h2],
                    in0=yb[:, :h2],
                    scalar=1.0,
                    in1=yb[:, h2:w],
                    op0=mult,
                    op1=addop,
                    accum_out=esum[:, idx : idx + 1],
                )

            def _dots_mul(lo, hi):
                # elementwise xf*g on the (otherwise idle) GPSIMD engine
                for t in range(lo, hi):
                    nc.gpsimd.tensor_tensor(
                        prod[:, t, :], xf_sb[:, t, :], g_sb[:, t, :], mult
                    )

            def _dots_sum(lo, hi):
                nc.vector.reduce_sum(
                    rawt[:, lo:hi], prod[:, lo:hi, :], axis=mybir.AxisListType.X
                )

            # ---- phase-3 machinery (runs per half so half 0 hides under
            # the stream and only half 1 is on the tail) ----
            esum_vh = [e.rearrange("p (t g) -> p t g", g=NCH) for e in esum_h]
            expn = stat.tile([128, NT], _F32)
            expt = stat.tile([128, NT], _F32)
            et = stat.tile([128, NT], _F32)
            denom = stat.tile([128, NT], _F32)
            ef = stat.tile([128, NT], _F32)
            mm = stat.tile([128, NT], _F32)
            acc = stat.tile([128, NT], _F32)
            L = stat.tile([128, NT], _F32)
            lsr = mybir.AluOpType.logical_shift_right
            band = mybir.AluOpType.bitwise_and
            bor = mybir.AluOpType.bitwise_or
            # ln(m) via degree-3 poly (max abs err 1.3e-3)
            PC = [
                1.0689890822e-01, -7.1197693854e-01, 2.0805856522e+00,
                -1.4741810531e+00,
            ]

            def _phase3(h):
                lo, hi = (0, NH) if h == 0 else (NH, NT)
                s = slice(lo, hi)
                nc.vector.reduce_sum(
                    et[:, s], esum_vh[h][:, :, :], axis=mybir.AxisListType.X
                )
                nc.vector.tensor_add(denom[:, s], et[:, s], expn[:, s])
                nc.vector.tensor_sub(denom[:, s], denom[:, s], expt[:, s])
                # ln(d) = ln2*e + p3(m), d = m * 2^e, m in [1,2)
                nc.vector.tensor_scalar(
                    acc[:, s].bitcast(_I32), denom[:, s].bitcast(_I32),
                    23, None, lsr,
                )
                nc.vector.tensor_scalar(
                    acc[:, s].bitcast(_I32), acc[:, s].bitcast(_I32),
                    127, None, sub,
                )
                nc.vector.tensor_copy(ef[:, s], acc[:, s].bitcast(_I32))
                nc.vector.tensor_scalar(
                    mm[:, s].bitcast(_I32), denom[:, s].bitcast(_I32),
                    0x7FFFFF, 0x3F800000, band, bor,
                )
                nc.vector.tensor_scalar(
                    acc[:, s], mm[:, s], PC[0], PC[1], mult, addop
                )
                nc.vector.tensor_mul(acc[:, s], acc[:, s], mm[:, s])
                nc.vector.tensor_scalar_add(acc[:, s], acc[:, s], PC[2])
                nc.vector.tensor_mul(acc[:, s], acc[:, s], mm[:, s])
                nc.vector.tensor_scalar_add(acc[:, s], acc[:, s], PC[3])
                nc.vector.scalar_tensor_tensor(
                    out=acc[:, s], in0=ef[:, s], scalar=LN2, in1=acc[:, s],
                    op0=mult, op1=addop,
                )
                nc.vector.tensor_sub(L[:, s], num[:, s], acc[:, s])
                nc.sync.dma_start(out=out_ext[:, s], in_=L[:, s])

            # ---- main stream: program order = per-engine schedule order ----
            for t in range(NT):
                chunks = _tile_layout(t)
                a_chunks = [x for x in chunks if x[0] == "A"]
                d_chunks = [x for x in chunks if x[0] == "D"]
                # interleave emission so the PE feeds both rings fairly
                ci = 0
                for j in range(max(len(a_chunks), len(d_chunks))):
                    if j < len(a_chunks):
                        _, c0, w = a_chunks[j]
                        _act_chunk(t, ci, c0, w)
                        ci += 1
                    if j < len(d_chunks):
                        _, c0, w = d_chunks[j]
                        _dve_chunk(t, ci, c0, w)
                        ci += 1
                if t == 1:
                    _dots_mul(0, NH)
                if t == 2:
                    _dots_sum(0, NH)
                    nc.vector.tensor_scalar_add(
                        num[:, :NH], rawt[:, :NH], -SM
                    )
                if t == 4:
                    nc.scalar.activation(expn[:, :NH], num[:, :NH], AF.Exp)
                    nc.scalar.activation(expt[:, :NH], rawt[:, :NH], AF.Exp)
                if t == 8:
                    _dots_mul(NH, NT)
                if t == 9:
                    _dots_sum(NH, NT)
                    nc.vector.tensor_scalar_add(
                        num[:, NH:], rawt[:, NH:], -SM
                    )
                if t == 10:
                    _phase3(0)
                if t == 12:
                    nc.scalar.activation(expn[:, NH:], num[:, NH:], AF.Exp)
                    nc.scalar.activation(expt[:, NH:], rawt[:, NH:], AF.Exp)
            _phase3(1)

    nc.finalize()
    return nc


_NC_CACHE = None


def _get_nc():
    global _NC_CACHE
    if _NC_CACHE is None:
        _NC_CACHE = _build_nc()
    return _NC_CACHE


def _shuffle_pm(a, nt):
    """[nt*128, d] row-major -> [128, nt, d] partition-major."""
    d = a.shape[-1]
    return np.ascontiguousarray(a.reshape(nt, 128, d).transpose(1, 0, 2))


def prep_core(xs, ls, W, wt=None):
    """Build one core's input map from its (pre-scaled) row block."""
    nt = xs.shape[0] // 128
    if wt is None:
        wt = _shuffle_pm(
            np.ascontiguousarray((WSCALE * W).T), KT
        ).astype(ml_dtypes.float8_e4m3)
    xt = _shuffle_pm(np.ascontiguousarray(xs.T), KT).astype(ml_dtypes.float8_e4m3)
    xf = _shuffle_pm(xs, nt).astype(ml_dtypes.bfloat16)
    g = _shuffle_pm(W[ls], nt).astype(ml_dtypes.bfloat16)
    return {"xt": xt, "wt": wt, "xf": xf, "g": g}


def make_in_maps(x, labels, W):
    x = np.asarray(x, dtype=np.float32)
    W = np.asarray(W, dtype=np.float32)
    labels = np.asarray(labels)
    # fold S / ||x_i|| into the embeddings on the host
    xs = x * (S / np.linalg.norm(x, axis=1, keepdims=True))
    wt = _shuffle_pm(
        np.ascontiguousarray((WSCALE * W).T), KT
    ).astype(ml_dtypes.float8_e4m3)
    return [
        prep_core(
            xs[i * NS : (i + 1) * NS], labels[i * NS : (i + 1) * NS], W, wt
        )
        for i in range(NCORES)
    ]


def run_device(x, labels, W, **kwargs):
    nc = _get_nc()
    in_maps = make_in_maps(x, labels, W)
    res = run_bass_kernel_spmd(nc, in_maps, list(range(NCORES)), **kwargs)
    return res


def finish(res):
    parts = []
    for i in range(NCORES):
        o = res.results[i]["out"]            # [128, NT]; row = t*128 + p
        parts.append(np.asarray(o).T.reshape(-1))
    L = np.concatenate(parts)
    return np.asarray(-np.mean(L), dtype=np.float32)


def kernel(x, labels, W):
    return finish(run_device(x, labels, W))


# revision 21
# speedup vs baseline: 1.0466x; 1.0466x over previous
"""AdMSoftmaxLoss fused distributed kernel for 8 TRN2 NeuronCores (v3).

Math (reference):
    xn = x / ||x||                     # row-L2-normalized embeddings
    wf = xn @ W.T                      # [N, C] logits
    tgt = wf[i, y_i]
    num = S * (tgt - M)
    excl = sum_c exp(S*wf) - exp(S*tgt)
    L = num - log(exp(num) + excl);  loss = -mean(L)

Strategy: pure data-parallel over N (2048 rows/core), no collectives.
The scale S/||x_i|| is folded into x on the HOST (xs = S*x/||x||), so the
device matmul produces final logits directly and needs no per-row scale.
  - PE: fp8e4 DoubleRow matmuls (K=256 per instruction; W pre-scaled by 16
    on the host for fp8 range, the 1/16 folded into the consumers).
  - The exp+row-sum work (20.5M elements/core) is SPLIT between the two
    engines that can read PSUM, each with its own PRIVATE PSUM ring so the
    rings self-pipeline with no cross-engine coupling:
      * ScalarE (ACT): exp activation with scale=1/16 and accum_out over
        double-buffered [128,1536] slots (6 banks);
      * VectorE (DVE): Schraudolph bit-trick exp over one [128,1024] slot
        (2 banks): tensor_scalar affine fp32(PSUM) -> int16 bf16-bits
        (round-to-nearest), then one scalar_tensor_tensor that adds the
        two bf16 halves and accum-sums the fp32 row total while the PE
        refills the slot.
    Schraudolph bf16 has ~+-4% sawtooth error, bias-corrected via the
    magic constant; per-row sums average it to ~0.3%, negligible vs the
    2e-2 gate.
  - Target logits S*tgt come from DVE dot products of bf16 xs rows with
    the host-gathered g = W[labels] rows (accum_out).
  - Final log via the DVE exponent/poly bit-trick (no ACT table reload),
    done per tile-half so only half the tail is exposed.
Per-row L values are DMA'd out; the host concatenates and means.
"""

import numpy as np
import ml_dtypes

import concourse.mybir as mybir
import concourse.tile as tile
from concourse import bacc
from concourse.bass_utils import run_bass_kernel_spmd

N, D, C = 16384, 256, 10000
S, M = 30.0, 0.4
NCORES = 8
NS = N // NCORES      # 2048 rows per core
NT = NS // 128        # 16 n-tiles of 128 rows
KT = D // 128         # 2 k-slices (one DoubleRow pass)

_F32 = mybir.dt.float32
_BF16 = mybir.dt.bfloat16
_I16 = mybir.dt.int16
_I32 = mybir.dt.int32
_F8 = mybir.dt.float8e4

LN2 = float(np.log(2.0))
WSCALE = 16.0                       # host pre-scale on W for fp8 range
A16 = 128.0 / LN2 / WSCALE          # Schraudolph slope on 16x logits
B16 = 16256.0 - 7.37                # bf16 magic offset, mean-unbiased
SM = S * M

N_WARMUP_MM = 10
AW = 1536                           # ACT ring slot width (3 banks x 2 bufs)
DW = 1024                           # DVE ring slot width (2 banks x 1 buf)

# Per-tile chunk layouts: (engine, col0, width) with 8 chunks per tile.
_LAYOUT_D4 = (
    [("A", 0, 1536), ("A", 1536, 1536), ("A", 3072, 1536), ("A", 4608, 1296)]
    + [("D", 5904 + i * 1024, 1024) for i in range(4)]
)
_LAYOUT_D3 = (
    [("A", 0, 1536), ("A", 1536, 1536), ("A", 3072, 1536), ("A", 4608, 1536),
     ("A", 6144, 784)]
    + [("D", 6928 + i * 1024, 1024) for i in range(3)]
)
# Head tiles: DVE columns sit right after A0 so the first DVE fills only
# need the first two wt DMA pieces (cols < 5632) and the DVE stream can
# start ~15us earlier.
_LAYOUT_HEAD = (
    [("A", 0, 1536)]
    + [("D", 1536 + i * 1024, 1024) for i in range(4)]
    + [("A", 5632, 1536), ("A", 7168, 1536), ("A", 8704, 1296)]
)
NCH = 8                             # chunks (esum slots) per tile


def _tile_layout(t):
    """5904/4096 (4 DVE chunks) or 6928/3072 (3 DVE chunks) col split."""
    if t < 2:
        return _LAYOUT_HEAD
    return _LAYOUT_D3 if (t % 8 in (2, 5, 7) or t == 6) else _LAYOUT_D4


def _build_nc(ns=NS, c=C):
    nt = ns // 128
    nc = bacc.Bacc("TRN2", target_bir_lowering=False)
    AF = mybir.ActivationFunctionType
    NT, C = nt, c  # noqa: N806
    NS = ns  # noqa: N806
    NH = NT // 2  # noqa: N806
    DR = mybir.MatmulPerfMode.DoubleRow  # noqa: N806
    mult = mybir.AluOpType.mult
    sub = mybir.AluOpType.subtract
    addop = mybir.AluOpType.add

    xt_ext = nc.declare_dram_parameter("xt", [128, KT, NS], _F8, isOutput=False)
    wt_ext = nc.declare_dram_parameter("wt", [128, KT, C], _F8, isOutput=False)
    xf_ext = nc.declare_dram_parameter("xf", [128, NT, D], _BF16, isOutput=False)
    g_ext = nc.declare_dram_parameter("g", [128, NT, D], _BF16, isOutput=False)
    out_ext = nc.declare_dram_parameter("out", [128, NT], _F32, isOutput=True)

    with tile.TileContext(nc) as tc:
        with (
            tc.tile_pool(name="big", bufs=1) as big,
            tc.tile_pool(name="stat", bufs=1) as stat,
            tc.tile_pool(name="scr", bufs=1) as scr,
            tc.tile_pool(name="expb", bufs=4) as expb,
            tc.tile_pool(name="ybuf", bufs=3) as ybuf,
            tc.tile_pool(name="dsum", bufs=2) as dsum,
            tc.tile_pool(name="psa", bufs=2, space="PSUM") as psa,
            tc.tile_pool(name="psd", bufs=1, space="PSUM") as psd,
        ):
            # ---- prologue: warm the exp ACT table + PE pstate during DMAs ----
            wu_a = scr.tile([128, KT, 128], _F8)
            wu_b = scr.tile([128, KT, 512], _F8)
            wu_e = scr.tile([128, 1], _F32)
            nc.gpsimd.memset(wu_a, 0.0)
            nc.gpsimd.memset(wu_b, 0.0)
            nc.gpsimd.memset(wu_e, 0.0)
            nc.scalar.activation(wu_e, wu_e, AF.Exp)  # pull exp table load
            wu_p = psa.tile([128, AW], _F32, tag="pa")
            for i in range(N_WARMUP_MM):
                nc.tensor.matmul(
                    wu_p[:, (i % 3) * 512 : (i % 3) * 512 + 512],
                    wu_a,
                    wu_b,
                    start=True,
                    stop=True,
                    perf_mode=DR,
                )

            # ---- input DMAs, ordered by when they gate compute ----
            xf_sb = big.tile([128, NT, D], _BF16)
            g_sb = big.tile([128, NT, D], _BF16)
            wt_sb = big.tile([128, KT, C], _F8)
            xt_sb = big.tile([128, KT, NS], _F8)

            def _wt_chunk(c0, w):
                for k in range(KT):
                    nc.sync.dma_start(
                        out=wt_sb[:, k, c0 : c0 + w], in_=wt_ext[:, k, c0 : c0 + w]
                    )

            _wt_chunk(0, 2048)
            for k in range(KT):
                nc.sync.dma_start(out=xt_sb[:, k, :], in_=xt_ext[:, k, :])
            _wt_chunk(2048, 2048)
            nc.sync.dma_start(out=xf_sb[:, :NH, :], in_=xf_ext[:, :NH, :])
            _wt_chunk(4096, 2048)
            _wt_chunk(6144, 2048)
            nc.sync.dma_start(out=g_sb[:, :NH, :], in_=g_ext[:, :NH, :])
            _wt_chunk(8192, 1808)
            nc.sync.dma_start(out=xf_sb[:, NH:, :], in_=xf_ext[:, NH:, :])
            nc.sync.dma_start(out=g_sb[:, NH:, :], in_=g_ext[:, NH:, :])

            # ---- per-(tile, chunk) partial exp-sums ----
            esum_a = stat.tile([128, NH * NCH], _F32)
            esum_b = stat.tile([128, NH * NCH], _F32)
            esum_h = [esum_a, esum_b]

            rawt = stat.tile([128, NT], _F32)   # S * tgt
            num = stat.tile([128, NT], _F32)    # S * (tgt - M)
            dotscr = scr.tile([128, D], _BF16)  # STT main-out scratch

            def _slot(t, ci):
                h, th = (0, t) if t < NH else (1, t - NH)
                return esum_h[h], th * NCH + ci

            def _fill(t, c0, w, pool, tag, width):
                pt = pool.tile([128, width], _F32, tag=tag)
                for b0 in range(0, w, 512):
                    bw = min(512, w - b0)
                    nc.tensor.matmul(
                        pt[:, b0 : b0 + bw],
                        xt_sb[:, :, t * 128 : (t + 1) * 128],
                        wt_sb[:, :, c0 + b0 : c0 + b0 + bw],
                        start=True,
                        stop=True,
                        perf_mode=DR,
                    )
                return pt

            def _act_chunk(t, ci, c0, w):
                pt = _fill(t, c0, w, psa, "pa", AW)
                esum, idx = _slot(t, ci)
                eo = expb.tile([128, AW], _BF16, tag="eo")
                nc.scalar.activation(
                    eo[:, :w],
                    pt[:, :w],
                    AF.Exp,
                    scale=1.0 / WSCALE,
                    accum_out=esum[:, idx : idx + 1],
                )

            def _dve_chunk(t, ci, c0, w):
                pt = _fill(t, c0, w, psd, "pd", DW)
                esum, idx = _slot(t, ci)
                y = ybuf.tile([128, DW], _I16, tag="y")
                # pass 1: i16 = rne(A16 * z16 + B16); bitcast(i16) ~ exp(z)
                nc.vector.tensor_scalar(
                    y[:, :w], pt[:, :w], A16, B16, mult, addop
                )
                yb = y.bitcast(_BF16)
                h2 = w // 2
                ds = dsum.tile([128, DW // 2], _BF16, tag="ds")
                # pass 2: halves-add + accumulate the fp32 row sum; the PE
                # refills the (already released) slot under this op
                nc.vector.scalar_tensor_tensor(
                    out=ds[:, :h2],
                    in0=yb[:, :h2],
                    scalar=1.0,
                    in1=yb[:, h2:w],
                    op0=mult,
                    op1=addop,
                    accum_out=esum[:, idx : idx + 1],
                )

            def _dots(lo, hi):
                for t in range(lo, hi):
                    nc.vector.scalar_tensor_tensor(
                        out=dotscr,
                        in0=xf_sb[:, t, :],
                        scalar=1.0,
                        in1=g_sb[:, t, :],
                        op0=mult,
                        op1=mult,
                        accum_out=rawt[:, t : t + 1],
                    )

            # ---- phase-3 machinery (runs per half so half 0 hides under
            # the stream and only half 1 is on the tail) ----
            esum_vh = [e.rearrange("p (t g) -> p t g", g=NCH) for e in esum_h]
            expn = stat.tile([128, NT], _F32)
            expt = stat.tile([128, NT], _F32)
            et = stat.tile([128, NT], _F32)
            denom = stat.tile([128, NT], _F32)
            ef = stat.tile([128, NT], _F32)
            mm = stat.tile([128, NT], _F32)
            acc = stat.tile([128, NT], _F32)
            L = stat.tile([128, NT], _F32)
            lsr = mybir.AluOpType.logical_shift_right
            band = mybir.AluOpType.bitwise_and
            bor = mybir.AluOpType.bitwise_or
            # ln(m) via degree-3 poly (max abs err 1.3e-3)
            PC = [
                1.0689890822e-01, -7.1197693854e-01, 2.0805856522e+00,
                -1.4741810531e+00,
            ]

            def _phase3(h):
                lo, hi = (0, NH) if h == 0 else (NH, NT)
                s = slice(lo, hi)
                nc.vector.reduce_sum(
                    et[:, s], esum_vh[h][:, :, :], axis=mybir.AxisListType.X
                )
                nc.vector.tensor_add(denom[:, s], et[:, s], expn[:, s])
                nc.vector.tensor_sub(denom[:, s], denom[:, s], expt[:, s])
                # ln(d) = ln2*e + p3(m), d = m * 2^e, m in [1,2)
                nc.vector.tensor_scalar(
                    acc[:, s].bitcast(_I32), denom[:, s].bitcast(_I32),
                    23, None, lsr,
                )
                nc.vector.tensor_scalar(
                    acc[:, s].bitcast(_I32), acc[:, s].bitcast(_I32),
                    127, None, sub,
                )
                nc.vector.tensor_copy(ef[:, s], acc[:, s].bitcast(_I32))
                nc.vector.tensor_scalar(
                    mm[:, s].bitcast(_I32), denom[:, s].bitcast(_I32),
                    0x7FFFFF, 0x3F800000, band, bor,
                )
                nc.vector.tensor_scalar(
                    acc[:, s], mm[:, s], PC[0], PC[1], mult, addop
                )
                nc.vector.tensor_mul(acc[:, s], acc[:, s], mm[:, s])
                nc.vector.tensor_scalar_add(acc[:, s], acc[:, s], PC[2])
                nc.vector.tensor_mul(acc[:, s], acc[:, s], mm[:, s])
                nc.vector.tensor_scalar_add(acc[:, s], acc[:, s], PC[3])
                nc.vector.scalar_tensor_tensor(
                    out=acc[:, s], in0=ef[:, s], scalar=LN2, in1=acc[:, s],
                    op0=mult, op1=addop,
                )
                nc.vector.tensor_sub(L[:, s], num[:, s], acc[:, s])
                nc.sync.dma_start(out=out_ext[:, s], in_=L[:, s])

            # ---- main stream: program order = per-engine schedule order ----
            for t in range(NT):
                chunks = _tile_layout(t)
                a_chunks = [x for x in chunks if x[0] == "A"]
                d_chunks = [x for x in chunks if x[0] == "D"]
                # interleave emission so the PE feeds both rings fairly
                ci = 0
                for j in range(max(len(a_chunks), len(d_chunks))):
                    if j < len(a_chunks):
                        _, c0, w = a_chunks[j]
                        _act_chunk(t, ci, c0, w)
                        ci += 1
                    if j < len(d_chunks):
                        _, c0, w = d_chunks[j]
                        _dve_chunk(t, ci, c0, w)
                        ci += 1
                if t == 1:
                    _dots(0, NH)
                    nc.vector.tensor_scalar_add(
                        num[:, :NH], rawt[:, :NH], -SM
                    )
                if t == 3:
                    nc.scalar.activation(expn[:, :NH], num[:, :NH], AF.Exp)
                    nc.scalar.activation(expt[:, :NH], rawt[:, :NH], AF.Exp)
                if t == 8:
                    _dots(NH, NT)
                    nc.vector.tensor_scalar_add(
                        num[:, NH:], rawt[:, NH:], -SM
                    )
                if t == 10:
                    _phase3(0)
                if t == 12:
                    nc.scalar.activation(expn[:, NH:], num[:, NH:], AF.Exp)
                    nc.scalar.activation(expt[:, NH:], rawt[:, NH:], AF.Exp)
            _phase3(1)

    nc.finalize()
    return nc


_NC_CACHE = None


def _get_nc():
    global _NC_CACHE
    if _NC_CACHE is None:
        _NC_CACHE = _build_nc()
    return _NC_CACHE


def _shuffle_pm(a, nt):
    """[nt*128, d] row-major -> [128, nt, d] partition-major."""
    d = a.shape[-1]
    return np.ascontiguousarray(a.reshape(nt, 128, d).transpose(1, 0, 2))


def prep_core(xs, ls, W, wt=None):
    """Build one core's input map from its (pre-scaled) row block."""
    nt = xs.shape[0] // 128
    if wt is None:
        wt = _shuffle_pm(
            np.ascontiguousarray((WSCALE * W).T), KT
        ).astype(ml_dtypes.float8_e4m3)
    xt = _shuffle_pm(np.ascontiguousarray(xs.T), KT).astype(ml_dtypes.float8_e4m3)
    xf = _shuffle_pm(xs, nt).astype(ml_dtypes.bfloat16)
    g = _shuffle_pm(W[ls], nt).astype(ml_dtypes.bfloat16)
    return {"xt": xt, "wt": wt, "xf": xf, "g": g}


def make_in_maps(x, labels, W):
    x = np.asarray(x, dtype=np.float32)
    W = np.asarray(W, dtype=np.float32)
    labels = np.asarray(labels)
    # fold S / ||x_i|| into the embeddings on the host
    xs = x * (S / np.linalg.norm(x, axis=1, keepdims=True))
    wt = _shuffle_pm(
        np.ascontiguousarray((WSCALE * W).T), KT
    ).astype(ml_dtypes.float8_e4m3)
    return [
        prep_core(
            xs[i * NS : (i + 1) * NS], labels[i * NS : (i + 1) * NS], W, wt
        )
        for i in range(NCORES)
    ]


def run_device(x, labels, W, **kwargs):
    nc = _get_nc()
    in_maps = make_in_maps(x, labels, W)
    res = run_bass_kernel_spmd(nc, in_maps, list(range(NCORES)), **kwargs)
    return res


def finish(res):
    parts = []
    for i in range(NCORES):
        o = res.results[i]["out"]            # [128, NT]; row = t*128 + p
        parts.append(np.asarray(o).T.reshape(-1))
    L = np.concatenate(parts)
    return np.asarray(-np.mean(L), dtype=np.float32)


def kernel(x, labels, W):
    return finish(run_device(x, labels, W))


# revision 22
# speedup vs baseline: 1.2496x; 1.1940x over previous
"""AdMSoftmaxLoss fused distributed kernel for 8 TRN2 NeuronCores (v3).

Math (reference):
    xn = x / ||x||                     # row-L2-normalized embeddings
    wf = xn @ W.T                      # [N, C] logits
    tgt = wf[i, y_i]
    num = S * (tgt - M)
    excl = sum_c exp(S*wf) - exp(S*tgt)
    L = num - log(exp(num) + excl);  loss = -mean(L)

Strategy: pure data-parallel over N (2048 rows/core), no collectives.
The scale S/||x_i|| is folded into x on the HOST (xs = S*x/||x||), so the
device matmul produces final logits directly and needs no per-row scale.
  - PE: fp8e4 DoubleRow matmuls (K=256 per instruction; W pre-scaled by 16
    on the host for fp8 range, the 1/16 folded into the consumers).
  - The exp+row-sum work (20.5M elements/core) is SPLIT between the two
    engines that can read PSUM, each with its own PRIVATE PSUM ring so the
    rings self-pipeline with no cross-engine coupling:
      * ScalarE (ACT): exp activation with scale=1/16 and accum_out over
        double-buffered [128,1536] slots (6 banks);
      * VectorE (DVE): Schraudolph bit-trick exp over one [128,1024] slot
        (2 banks): tensor_scalar affine fp32(PSUM) -> int16 bf16-bits
        (round-to-nearest), then one scalar_tensor_tensor that adds the
        two bf16 halves and accum-sums the fp32 row total while the PE
        refills the slot.
    Schraudolph bf16 has ~+-4% sawtooth error, bias-corrected via the
    magic constant; per-row sums average it to ~0.3%, negligible vs the
    2e-2 gate.
  - Target logits S*tgt come from DVE dot products of bf16 xs rows with
    the host-gathered g = W[labels] rows (accum_out).
  - Final log via the DVE exponent/poly bit-trick (no ACT table reload),
    done per tile-half so only half the tail is exposed.
Per-row L values are DMA'd out; the host concatenates and means.
"""

import numpy as np
import ml_dtypes

import concourse.mybir as mybir
import concourse.tile as tile
from concourse import bacc
from concourse.bass_utils import run_bass_kernel_spmd

N, D, C = 16384, 256, 10000
S, M = 30.0, 0.4
NCORES = 8
NS = N // NCORES      # 2048 rows per core
NT = NS // 128        # 16 n-tiles of 128 rows
KT = D // 128         # 2 k-slices (one DoubleRow pass)

_F32 = mybir.dt.float32
_BF16 = mybir.dt.bfloat16
_I16 = mybir.dt.int16
_I32 = mybir.dt.int32
_F8 = mybir.dt.float8e4

LN2 = float(np.log(2.0))
WSCALE = 16.0                       # host pre-scale on W for fp8 range
A16 = 128.0 / LN2 / WSCALE          # Schraudolph slope on 16x logits
B16 = 16256.0 - 7.37                # bf16 magic offset, mean-unbiased
SM = S * M

N_WARMUP_MM = 10
AW = 1536                           # ACT ring slot width (3 banks x 2 bufs)
DW = 1024                           # DVE ring slot width (2 banks x 1 buf)

# Per-tile chunk layouts: (engine, col0, width) with 8 chunks per tile.
_LAYOUT_D4 = (
    [("A", 0, 1536), ("A", 1536, 1536), ("A", 3072, 1536), ("A", 4608, 1296)]
    + [("D", 5904 + i * 1024, 1024) for i in range(4)]
)
_LAYOUT_D3 = (
    [("A", 0, 1536), ("A", 1536, 1536), ("A", 3072, 1536), ("A", 4608, 1536),
     ("A", 6144, 784)]
    + [("D", 6928 + i * 1024, 1024) for i in range(3)]
)
# Head tiles: DVE columns sit right after A0 so the first DVE fills only
# need the first two wt DMA pieces (cols < 5632) and the DVE stream can
# start ~15us earlier.
_LAYOUT_HEAD = (
    [("A", 0, 1536)]
    + [("D", 1536 + i * 1024, 1024) for i in range(4)]
    + [("A", 5632, 1536), ("A", 7168, 1536), ("A", 8704, 1296)]
)
NCH = 8                             # chunks (esum slots) per tile


def _tile_layout(t):
    """5904/4096 (4 DVE chunks) or 6928/3072 (3 DVE chunks) col split."""
    if t < 2:
        return _LAYOUT_HEAD
    return _LAYOUT_D3 if t % 8 in (2, 5, 7) else _LAYOUT_D4


def _build_nc(ns=NS, c=C):
    nt = ns // 128
    nc = bacc.Bacc("TRN2", target_bir_lowering=False)
    AF = mybir.ActivationFunctionType
    NT, C = nt, c  # noqa: N806
    NS = ns  # noqa: N806
    NH = NT // 2  # noqa: N806
    DR = mybir.MatmulPerfMode.DoubleRow  # noqa: N806
    mult = mybir.AluOpType.mult
    sub = mybir.AluOpType.subtract
    addop = mybir.AluOpType.add

    xt_ext = nc.declare_dram_parameter("xt", [128, KT, NS], _F8, isOutput=False)
    wt_ext = nc.declare_dram_parameter("wt", [128, KT, C], _F8, isOutput=False)
    xf_ext = nc.declare_dram_parameter("xf", [128, NT, D], _BF16, isOutput=False)
    g_ext = nc.declare_dram_parameter("g", [128, NT, D], _BF16, isOutput=False)
    out_ext = nc.declare_dram_parameter("out", [128, NT], _F32, isOutput=True)

    with tile.TileContext(nc) as tc:
        with (
            tc.tile_pool(name="big", bufs=1) as big,
            tc.tile_pool(name="stat", bufs=1) as stat,
            tc.tile_pool(name="scr", bufs=1) as scr,
            tc.tile_pool(name="expb", bufs=4) as expb,
            tc.tile_pool(name="ybuf", bufs=3) as ybuf,
            tc.tile_pool(name="dsum", bufs=2) as dsum,
            tc.tile_pool(name="psa", bufs=2, space="PSUM") as psa,
            tc.tile_pool(name="psd", bufs=1, space="PSUM") as psd,
        ):
            # ---- prologue: warm the exp ACT table + PE pstate during DMAs ----
            wu_a = scr.tile([128, KT, 128], _F8)
            wu_b = scr.tile([128, KT, 512], _F8)
            wu_e = scr.tile([128, 1], _F32)
            nc.gpsimd.memset(wu_a, 0.0)
            nc.gpsimd.memset(wu_b, 0.0)
            nc.gpsimd.memset(wu_e, 0.0)
            nc.scalar.activation(wu_e, wu_e, AF.Exp)  # pull exp table load
            wu_p = psa.tile([128, AW], _F32, tag="pa")
            for i in range(N_WARMUP_MM):
                nc.tensor.matmul(
                    wu_p[:, (i % 3) * 512 : (i % 3) * 512 + 512],
                    wu_a,
                    wu_b,
                    start=True,
                    stop=True,
                    perf_mode=DR,
                )

            # ---- input DMAs, ordered by when they gate compute ----
            xf_sb = big.tile([128, NT, D], _BF16)
            g_sb = big.tile([128, NT, D], _BF16)
            wt_sb = big.tile([128, KT, C], _F8)
            xt_sb = big.tile([128, KT, NS], _F8)

            def _wt_chunk(c0, w):
                for k in range(KT):
                    nc.sync.dma_start(
                        out=wt_sb[:, k, c0 : c0 + w], in_=wt_ext[:, k, c0 : c0 + w]
                    )

            _wt_chunk(0, 2048)
            for k in range(KT):
                nc.sync.dma_start(out=xt_sb[:, k, :], in_=xt_ext[:, k, :])
            _wt_chunk(2048, 2048)
            nc.sync.dma_start(out=xf_sb[:, :NH, :], in_=xf_ext[:, :NH, :])
            _wt_chunk(4096, 2048)
            _wt_chunk(6144, 2048)
            nc.sync.dma_start(out=g_sb[:, :NH, :], in_=g_ext[:, :NH, :])
            _wt_chunk(8192, 1808)
            nc.sync.dma_start(out=xf_sb[:, NH:, :], in_=xf_ext[:, NH:, :])
            nc.sync.dma_start(out=g_sb[:, NH:, :], in_=g_ext[:, NH:, :])

            # ---- per-(tile, chunk) partial exp-sums ----
            esum_a = stat.tile([128, NH * NCH], _F32)
            esum_b = stat.tile([128, NH * NCH], _F32)
            esum_h = [esum_a, esum_b]

            rawt = stat.tile([128, NT], _F32)   # S * tgt
            num = stat.tile([128, NT], _F32)    # S * (tgt - M)
            dotscr = scr.tile([128, D], _BF16)  # STT main-out scratch

            def _slot(t, ci):
                h, th = (0, t) if t < NH else (1, t - NH)
                return esum_h[h], th * NCH + ci

            def _fill(t, c0, w, pool, tag, width):
                pt = pool.tile([128, width], _F32, tag=tag)
                for b0 in range(0, w, 512):
                    bw = min(512, w - b0)
                    nc.tensor.matmul(
                        pt[:, b0 : b0 + bw],
                        xt_sb[:, :, t * 128 : (t + 1) * 128],
                        wt_sb[:, :, c0 + b0 : c0 + b0 + bw],
                        start=True,
                        stop=True,
                        perf_mode=DR,
                    )
                return pt

            def _act_chunk(t, ci, c0, w):
                pt = _fill(t, c0, w, psa, "pa", AW)
                esum, idx = _slot(t, ci)
                eo = expb.tile([128, AW], _BF16, tag="eo")
                nc.scalar.activation(
                    eo[:, :w],
                    pt[:, :w],
                    AF.Exp,
                    scale=1.0 / WSCALE,
                    accum_out=esum[:, idx : idx + 1],
                )

            def _dve_chunk(t, ci, c0, w):
                pt = _fill(t, c0, w, psd, "pd", DW)
                esum, idx = _slot(t, ci)
                y = ybuf.tile([128, DW], _I16, tag="y")
                # pass 1: i16 = rne(A16 * z16 + B16); bitcast(i16) ~ exp(z)
                nc.vector.tensor_scalar(
                    y[:, :w], pt[:, :w], A16, B16, mult, addop
                )
                yb = y.bitcast(_BF16)
                h2 = w // 2
                ds = dsum.tile([128, DW // 2], _BF16, tag="ds")
                # pass 2: halves-add + accumulate the fp32 row sum; the PE
                # refills the (already released) slot under this op
                nc.vector.scalar_tensor_tensor(
                    out=ds[:, :h2],
                    in0=yb[:, :h2],
                    scalar=1.0,
                    in1=yb[:, h2:w],
                    op0=mult,
                    op1=addop,
                    accum_out=esum[:, idx : idx + 1],
                )

            def _dots(lo, hi):
                for t in range(lo, hi):
                    nc.vector.scalar_tensor_tensor(
                        out=dotscr,
                        in0=xf_sb[:, t, :],
                        scalar=1.0,
                        in1=g_sb[:, t, :],
                        op0=mult,
                        op1=mult,
                        accum_out=rawt[:, t : t + 1],
                    )

            # ---- phase-3 machinery (runs per half so half 0 hides under
            # the stream and only half 1 is on the tail) ----
            esum_vh = [e.rearrange("p (t g) -> p t g", g=NCH) for e in esum_h]
            expn = stat.tile([128, NT], _F32)
            expt = stat.tile([128, NT], _F32)
            et = stat.tile([128, NT], _F32)
            denom = stat.tile([128, NT], _F32)
            ef = stat.tile([128, NT], _F32)
            mm = stat.tile([128, NT], _F32)
            acc = stat.tile([128, NT], _F32)
            L = stat.tile([128, NT], _F32)
            lsr = mybir.AluOpType.logical_shift_right
            band = mybir.AluOpType.bitwise_and
            bor = mybir.AluOpType.bitwise_or
            # ln(m) via degree-3 poly (max abs err 1.3e-3)
            PC = [
                1.0689890822e-01, -7.1197693854e-01, 2.0805856522e+00,
                -1.4741810531e+00,
            ]

            def _phase3(h):
                lo, hi = (0, NH) if h == 0 else (NH, NT)
                s = slice(lo, hi)
                nc.vector.reduce_sum(
                    et[:, s], esum_vh[h][:, :, :], axis=mybir.AxisListType.X
                )
                nc.vector.tensor_add(denom[:, s], et[:, s], expn[:, s])
                nc.vector.tensor_sub(denom[:, s], denom[:, s], expt[:, s])
                # ln(d) = ln2*e + p3(m), d = m * 2^e, m in [1,2)
                nc.vector.tensor_scalar(
                    acc[:, s].bitcast(_I32), denom[:, s].bitcast(_I32),
                    23, None, lsr,
                )
                nc.vector.tensor_scalar(
                    acc[:, s].bitcast(_I32), acc[:, s].bitcast(_I32),
                    127, None, sub,
                )
                nc.vector.tensor_copy(ef[:, s], acc[:, s].bitcast(_I32))
                nc.vector.tensor_scalar(
                    mm[:, s].bitcast(_I32), denom[:, s].bitcast(_I32),
                    0x7FFFFF, 0x3F800000, band, bor,
                )
                nc.vector.tensor_scalar(
                    acc[:, s], mm[:, s], PC[0], PC[1], mult, addop
                )
                nc.vector.tensor_mul(acc[:, s], acc[:, s], mm[:, s])
                nc.vector.tensor_scalar_add(acc[:, s], acc[:, s], PC[2])
                nc.vector.tensor_mul(acc[:, s], acc[:, s], mm[:, s])
                nc.vector.tensor_scalar_add(acc[:, s], acc[:, s], PC[3])
                nc.vector.scalar_tensor_tensor(
                    out=acc[:, s], in0=ef[:, s], scalar=LN2, in1=acc[:, s],
                    op0=mult, op1=addop,
                )
                nc.vector.tensor_sub(L[:, s], num[:, s], acc[:, s])
                nc.sync.dma_start(out=out_ext[:, s], in_=L[:, s])

            # ---- main stream: program order = per-engine schedule order ----
            for t in range(NT):
                chunks = _tile_layout(t)
                a_chunks = [x for x in chunks if x[0] == "A"]
                d_chunks = [x for x in chunks if x[0] == "D"]
                # interleave emission so the PE feeds both rings fairly
                ci = 0
                for j in range(max(len(a_chunks), len(d_chunks))):
                    if j < len(a_chunks):
                        _, c0, w = a_chunks[j]
                        _act_chunk(t, ci, c0, w)
                        ci += 1
                    if j < len(d_chunks):
                        _, c0, w = d_chunks[j]
                        _dve_chunk(t, ci, c0, w)
                        ci += 1
                if t == 1:
                    _dots(0, NH)
                    nc.vector.tensor_scalar_add(
                        num[:, :NH], rawt[:, :NH], -SM
                    )
                if t == 3:
                    nc.scalar.activation(expn[:, :NH], num[:, :NH], AF.Exp)
                    nc.scalar.activation(expt[:, :NH], rawt[:, :NH], AF.Exp)
                if t == 8:
                    _dots(NH, NT)
                    nc.vector.tensor_scalar_add(
                        num[:, NH:], rawt[:, NH:], -SM
                    )
                if t == 10:
                    _phase3(0)
                if t == 12:
                    nc.scalar.activation(expn[:, NH:], num[:, NH:], AF.Exp)
                    nc.scalar.activation(expt[:, NH:], rawt[:, NH:], AF.Exp)
            _phase3(1)

    nc.finalize()
    return nc


_NC_CACHE = None


def _get_nc():
    global _NC_CACHE
    if _NC_CACHE is None:
        _NC_CACHE = _build_nc()
    return _NC_CACHE


def _shuffle_pm(a, nt):
    """[nt*128, d] row-major -> [128, nt, d] partition-major."""
    d = a.shape[-1]
    return np.ascontiguousarray(a.reshape(nt, 128, d).transpose(1, 0, 2))


def prep_core(xs, ls, W, wt=None):
    """Build one core's input map from its (pre-scaled) row block."""
    nt = xs.shape[0] // 128
    if wt is None:
        wt = _shuffle_pm(
            np.ascontiguousarray((WSCALE * W).T), KT
        ).astype(ml_dtypes.float8_e4m3)
    xt = _shuffle_pm(np.ascontiguousarray(xs.T), KT).astype(ml_dtypes.float8_e4m3)
    xf = _shuffle_pm(xs, nt).astype(ml_dtypes.bfloat16)
    g = _shuffle_pm(W[ls], nt).astype(ml_dtypes.bfloat16)
    return {"xt": xt, "wt": wt, "xf": xf, "g": g}


def make_in_maps(x, labels, W):
    x = np.asarray(x, dtype=np.float32)
    W = np.asarray(W, dtype=np.float32)
    labels = np.asarray(labels)
    # fold S / ||x_i|| into the embeddings on the host
    xs = x * (S / np.linalg.norm(x, axis=1, keepdims=True))
    wt = _shuffle_pm(
        np.ascontiguousarray((WSCALE * W).T), KT
    ).astype(ml_dtypes.float8_e4m3)
    return [
        prep_core(
            xs[i * NS : (i + 1) * NS], labels[i * NS : (i + 1) * NS], W, wt
        )
        for i in range(NCORES)
    ]


def run_device(x, labels, W, **kwargs):
    nc = _get_nc()
    in_maps = make_in_maps(x, labels, W)
    res = run_bass_kernel_spmd(nc, in_maps, list(range(NCORES)), **kwargs)
    return res


def finish(res):
    parts = []
    for i in range(NCORES):
        o = res.results[i]["out"]            # [128, NT]; row = t*128 + p
        parts.append(np.asarray(o).T.reshape(-1))
    L = np.concatenate(parts)
    return np.asarray(-np.mean(L), dtype=np.float32)


def kernel(x, labels, W):
    return finish(run_device(x, labels, W))
